# revision 42
# baseline (speedup 1.0000x reference)
"""Trainium2 Bass kernel for nn_CombinedMLPMoEModel (moe_routing).

Strategy (8 NeuronCores, pure data parallel on the batch):
 - Host: shard batch 16384 -> 8 x 2048 tokens, pre-transpose x1/x2/x3 to
   feature-major [Din, tok]; replicate weights.
 - The final output is a scalar per token: concat(o1,o2,o3) @ Wf -> bn
   -> @ Wr.  That tail is linear, so each MoE expert's contribution
   collapses to a per-token SCALAR:
       o_j . wfr_j = sum_k g_k (m_j . (W_e @ wfr_j) + b_e . wfr_j)
   with wfr = Wf @ (bn_scale * Wr).  The dense [512x512] expert matmuls
   disappear; per part we need one [512 -> 8] matmul (like the router),
   computed in exact fp32.
 - LayerNorm is linear per token, so it folds into those matmuls:
       logits = r*(t3 @ (lng.Wg)) - (mu*r)*c_g + bias_g     (same for S)
   i.e. no normalized tensor is ever materialized; the per-token affine
   (r, mu*r) is applied on [128,48] tiles.
 - The proj layer folds into W1 on the host (WF_i = Wp_i @ W1_i; same
   FLOPs, one less pipeline stage).
 - MLP chain precision: fp32r (tf32-like) 3-term split
       W @ x ~= Whi@xhi + Whi@xlo + Wlo@xhi
   with Whi/Wlo PRE-SPLIT ON THE HOST (doubles weight DMA, removes the
   on-chip weight-split engine work).  Routing (top-2 of 8) then matches
   the fp32 reference exactly (0 flips on the reference inputs; total
   rel err ~1e-5).
 - LN stats token-major: sum(t3) rides as a free ones-column (col 48) of
   the z-matmul; sum(t3^2) via a squares tensor against a ones vector.
"""

import numpy as np
import ml_dtypes
from contextlib import ExitStack

import concourse.bass as bass
from concourse import bacc
import concourse.mybir as mybir
import concourse.tile as tile
from concourse.bass_utils import run_bass_kernel_spmd

F32 = mybir.dt.float32
F32R = mybir.dt.float32r
F8 = mybir.dt.float8e5
DR = mybir.MatmulPerfMode.DoubleRow
AF = mybir.ActivationFunctionType
ALU = mybir.AluOpType
AX = mybir.AxisListType

N_CORES = 8
B = 16384
DIN = 1024
D = 512
H = 1024
D3 = 3 * D          # 1536
E = 8
TOK_CORE = B // N_CORES   # 2048
MT = 512                  # megatile tokens
EPS = 1e-5
NEG_BIG = -1.0e30

_PROGRAM_CACHE = {}


def _chunks(n):
    return n // 128


def build_program(n_tok=TOK_CORE, mt=MT):
    """Build the per-core Bass program (SPMD across the 8 cores)."""
    nc = bacc.Bacc(None, target_bir_lowering=False)
    T = n_tok // mt
    CS = mt
    NCH = _chunks(mt)          # 128-token chunks per megatile
    NCOL = T * NCH             # output columns per core

    # ---------------- DRAM I/O ----------------
    xs = [nc.dram_tensor(f"x{i+1}t", [DIN, n_tok], F32, kind="ExternalInput")
          for i in range(3)]
    WFhi = nc.dram_tensor("WFhi", [3 * DIN, H], F32R, kind="ExternalInput")
    W2hir = nc.dram_tensor("W2hir", [128, _chunks(H), H], F32R, kind="ExternalInput")
    W3hi = nc.dram_tensor("W3hi", [H, D3], F32R, kind="ExternalInput")
    # fp8-e5m2 lo-pass weights, packed as [128, kpair, 2, dout]
    WFh8d = nc.dram_tensor("WFh8", [128, 12, 2, H], F8, kind="ExternalInput")
    WFl8d = nc.dram_tensor("WFl8", [128, 12, 2, H], F8, kind="ExternalInput")
    W2h8d = nc.dram_tensor("W2h8", [128, 4, 2, H], F8, kind="ExternalInput")
    s1d = nc.dram_tensor("s1v", [128, _chunks(H)], F32, kind="ExternalInput")
    b1d = nc.dram_tensor("b1v", [128, _chunks(H)], F32, kind="ExternalInput")
    s2d = nc.dram_tensor("s2v", [128, _chunks(H)], F32, kind="ExternalInput")
    b2d = nc.dram_tensor("b2v", [128, _chunks(H)], F32, kind="ExternalInput")
    b3d = nc.dram_tensor("b3v", [128, _chunks(D3)], F32, kind="ExternalInput")
    wzhd = nc.dram_tensor("wzh", [128, _chunks(H), 49], F32, kind="ExternalInput")
    zbd = nc.dram_tensor("zb", [1, 49], F32, kind="ExternalInput")
    onesrd = nc.dram_tensor("ones_row", [1, 128], F32, kind="ExternalInput")
    cnegd = nc.dram_tensor("cneg", [1, 48], F32, kind="ExternalInput")
    bzd = nc.dram_tensor("bz", [1, 48], F32, kind="ExternalInput")
    onesd = nc.dram_tensor("ones_col", [128, 1], F32, kind="ExternalInput")
    out_d = nc.dram_tensor("out", [128, NCOL], F32, kind="ExternalOutput")

    with tile.TileContext(nc) as tc, ExitStack() as ctx:
        cp = ctx.enter_context(tc.tile_pool(name="consts", bufs=1))
        sp = ctx.enter_context(tc.tile_pool(name="work", bufs=1))
        ps = ctx.enter_context(tc.tile_pool(name="psum", bufs=8, space="PSUM"))

        def pt(shape, dtype, tag, bufs=None):
            return sp.tile(shape, dtype, tag=tag, bufs=bufs, name=tag)

        # ---------------- resident constants ----------------
        # consts ride the gpsimd/vector DMA queues so the first megatile's
        # x/weight DMAs on the sync queue are not delayed behind them
        wzh_sb = cp.tile([128, _chunks(H), 49], F32, name="wzh_sb")
        nc.scalar.dma_start(out=wzh_sb, in_=wzhd[:, :, :])
        # W2 weights are small enough to keep SBUF-resident for all megatiles
        w2r_sb = cp.tile([128, _chunks(H), H], F32R, name="w2r_sb")
        nc.scalar.dma_start(out=w2r_sb, in_=W2hir[:, :, :])
        w2h8_sb = cp.tile([128, 4, 2, H], F8, name="w2h8_sb")
        nc.scalar.dma_start(out=w2h8_sb, in_=W2h8d[:, :, :, :])
        zb_sb = cp.tile([1, 49], F32, name="zb_sb")
        nc.gpsimd.dma_start(out=zb_sb, in_=zbd[:, :])
        onesr_sb = cp.tile([1, 128], F32, name="onesr_sb")
        nc.gpsimd.dma_start(out=onesr_sb, in_=onesrd[:, :])
        ones_sb = cp.tile([128, 1], F32, name="ones_sb")
        nc.gpsimd.dma_start(out=ones_sb, in_=onesd[:, :])
        cneg_bc = cp.tile([128, 48], F32, name="cneg_bc")
        nc.gpsimd.dma_start(
            out=cneg_bc,
            in_=bass.AP(tensor=cnegd[:, :].tensor, offset=0, ap=[[0, 128], [1, 48]]),
        )
        bz_bc = cp.tile([128, 48], F32, name="bz_bc")
        nc.gpsimd.dma_start(
            out=bz_bc,
            in_=bass.AP(tensor=bzd[:, :].tensor, offset=0, ap=[[0, 128], [1, 48]]),
        )

        def ldvec(dram, nch, name):
            t = cp.tile([128, nch], F32, name=name)
            nc.gpsimd.dma_start(out=t, in_=dram[:, :])
            return t

        s1_sb = ldvec(s1d, _chunks(H), "s1_sb")
        b1_sb = ldvec(b1d, _chunks(H), "b1_sb")
        s2_sb = ldvec(s2d, _chunks(H), "s2_sb")
        b2_sb = ldvec(b2d, _chunks(H), "b2_sb")
        b3_sb = ldvec(b3d, _chunks(D3), "b3_sb")

        out128 = cp.tile([128, NCOL], F32, name="out128")

        def split_act(src_ap):
            """tf32 hi/lo split of one [128, CS] fp32 activation chunk."""
            hi = pt([128, CS], F32R, tag="aph", bufs=2)
            nc.scalar.copy(hi, src_ap)
            lo = pt([128, CS], F32R, tag="apl", bufs=2)
            nc.vector.scalar_tensor_tensor(out=lo, in0=src_ap, scalar=-1.0,
                                           in1=hi.bitcast(F32), op0=ALU.bypass,
                                           op1=ALU.subtract)
            return hi, lo

        def load_wh(hi_dram, r0, dgs, dgw, tag, q=nc.sync, bufs=4):
            """DMA pre-split tf32 hi weight chunk (rows [r0, r0+128),
            dout cols [dgs, dgs+dgw)). Per-layer tag so next layer's
            prefetch is not serialized on this layer's buffers."""
            wh = pt([128, 1024], F32R, tag=tag, bufs=bufs)[:, :dgw]
            q.dma_start(out=wh, in_=hi_dram[r0:r0 + 128, dgs:dgs + dgw])
            return wh

        def load_w8(dram8, kp, tag):
            w8 = pt([128, 2, H], F8, tag=tag, bufs=4)
            nc.scalar.dma_start(out=w8, in_=dram8[:, kp, :, :])
            return w8

        # ---------------- megatile loop ----------------
        for t in range(T):
            ts = slice(t * CS, (t + 1) * CS)

            # ---- W1F: h1 = relu(bn(sum_i x_i @ WF_i + b1')) ----
            # pass 1: tf32 Whi@xhi per k-chunk; passes 2+3: fp8-e5m2
            # DoubleRow over k-pairs (Whi@xlo and Wlo@xhi, scale-neutral).
            h1 = pt([128, _chunks(H), CS], F32, tag="h1", bufs=1)
            psums = [ps.tile([128, 512], F32, tag="mm", name="p1") for _ in range(8)]
            for kp in range(12):
                xh8 = pt([128, 2, CS], F8, tag="xh8", bufs=2)
                xl8 = pt([128, 2, CS], F8, tag="xl8", bufs=2)
                for kk in range(2):
                    kc = 2 * kp + kk
                    i, k = kc // _chunks(DIN), kc % _chunks(DIN)
                    xk = pt([128, CS], F32, tag="xk", bufs=3)
                    nc.gpsimd.dma_start(out=xk, in_=xs[i][128 * k:128 * (k + 1), ts])
                    xh, xl = split_act(xk)
                    nc.vector.tensor_scalar(out=xh8[:, kk, :], in0=xh.bitcast(F32),
                                            scalar1=1.0 / 1024.0, scalar2=None,
                                            op0=ALU.mult)
                    nc.vector.tensor_scalar(out=xl8[:, kk, :], in0=xl.bitcast(F32),
                                            scalar1=16.0, scalar2=None,
                                            op0=ALU.mult)
                    wh = load_wh(WFhi, 128 * kc, 0, 1024, "whf", bufs=5)
                    for d in range(8):
                        nc.tensor.matmul(psums[d], wh[:, 128 * d:128 * (d + 1)],
                                         xh, start=(kc == 0), stop=False)
                wh8 = load_w8(WFh8d, kp, "wfh8")
                wl8 = load_w8(WFl8d, kp, "wfl8")
                for d in range(8):
                    sl = slice(128 * d, 128 * (d + 1))
                    nc.tensor.matmul(psums[d], wh8[:, :, sl], xl8,
                                     start=False, stop=False, perf_mode=DR)
                    nc.tensor.matmul(psums[d], wl8[:, :, sl], xh8,
                                     start=False, stop=(kp == 11), perf_mode=DR)
            for d in range(8):
                nc.scalar.activation(h1[:, d, :], psums[d], AF.Relu,
                                     bias=b1_sb[:, d:d + 1], scale=s1_sb[:, d:d + 1])

            # ---- W2 -> h2: tf32 hi-pass + fp8 DoubleRow lo-pass ----
            h2 = pt([128, _chunks(H), CS], F32, tag="h2", bufs=1)
            psums = [ps.tile([128, 512], F32, tag="mm", name="p2") for _ in range(8)]
            for kp in range(4):
                al8 = pt([128, 2, CS], F8, tag="al8", bufs=2)
                for kk in range(2):
                    k = 2 * kp + kk
                    ah, al = split_act(h1[:, k, :])
                    nc.vector.tensor_scalar(out=al8[:, kk, :], in0=al.bitcast(F32),
                                            scalar1=16.0, scalar2=None,
                                            op0=ALU.mult)
                    for d in range(8):
                        nc.tensor.matmul(psums[d],
                                         w2r_sb[:, k, 128 * d:128 * (d + 1)],
                                         ah, start=(k == 0), stop=False)
                for d in range(8):
                    sl = slice(128 * d, 128 * (d + 1))
                    nc.tensor.matmul(psums[d], w2h8_sb[:, kp, :, sl], al8,
                                     start=False, stop=(kp == 3), perf_mode=DR)
            for d in range(8):
                nc.scalar.activation(h2[:, d, :], psums[d], AF.Relu,
                                     bias=b2_sb[:, d:d + 1], scale=s2_sb[:, d:d + 1])

            # ---- W3, single pass (feeds LN variance only); squares -> sqacc ----
            ahh = pt([128, _chunks(H), CS], F32R, tag="ahh", bufs=1)
            for k in range(_chunks(H)):
                nc.scalar.copy(ahh[:, k, :], h2[:, k, :])
            sqacc = pt([128, CS], F32, tag="sqa", bufs=2)
            for dg in range(2):
                psums = [ps.tile([128, 512], F32, tag="mm", name="p3")
                         for _ in range(6)]
                for k in range(_chunks(H)):
                    wh = load_wh(W3hi, 128 * k, 768 * dg, 768, "wh3", q=nc.scalar)
                    for d in range(6):
                        nc.tensor.matmul(psums[d], wh[:, 128 * d:128 * (d + 1)],
                                         ahh[:, k, :],
                                         start=(k == 0), stop=(k == _chunks(H) - 1))
                for d in range(6):
                    dd = 6 * dg + d
                    if dd == 0:
                        nc.scalar.activation(sqacc, psums[d], AF.Square,
                                             bias=b3_sb[:, dd:dd + 1], scale=1.0)
                    else:
                        sqt = pt([128, CS], F32, tag="sqt", bufs=2)
                        nc.scalar.activation(sqt, psums[d], AF.Square,
                                             bias=b3_sb[:, dd:dd + 1], scale=1.0)
                        nc.vector.tensor_add(sqacc, sqacc, sqt)

            # ---- tail: z entirely from h2 (exact W3 fold) per 128-chunk ----
            # pq shares pz's bank (col 49): keeps psum allocations/megatile at
            # a multiple of 8 so next megatile's W1F psums reuse long-freed
            # banks instead of waiting on the tail. pz's first matmul
            # (start=True) cleared the whole bank, so the col-49 write with
            # start=False lands fresh.
            for c in range(NCH):
                cs_ = slice(128 * c, 128 * (c + 1))
                pz = ps.tile([128, 50], F32, tag="mm", name="pz")
                for k in range(_chunks(H)):
                    nc.tensor.matmul(pz[:, 0:49], h2[:, k, cs_], wzh_sb[:, k, :],
                                     start=(k == 0), stop=False)
                # + [b3 @ wz | sum(b3)] broadcast row
                nc.tensor.matmul(pz[:, 0:49], onesr_sb, zb_sb,
                                 start=False, stop=True)
                nc.tensor.matmul(pz[:, 49:50], sqacc[:, cs_], ones_sb,
                                 start=False, stop=True, skip_group_check=True)

                # per-token LN stats (token-major [128,1])
                mu = pt([128, 1], F32, tag="mu", bufs=2)
                nc.vector.tensor_scalar(out=mu, in0=pz[:, 48:49],
                                        scalar1=1.0 / D3, scalar2=None, op0=ALU.mult)
                et2 = pt([128, 1], F32, tag="et2", bufs=2)
                nc.vector.tensor_scalar(out=et2, in0=pz[:, 49:50], scalar1=1.0 / D3,
                                        scalar2=EPS, op0=ALU.mult, op1=ALU.add)
                msq = pt([128, 1], F32, tag="msq", bufs=2)
                nc.scalar.activation(msq, mu, AF.Square)
                veps = pt([128, 1], F32, tag="veps", bufs=2)
                nc.vector.scalar_tensor_tensor(out=veps, in0=msq, scalar=-1.0,
                                               in1=et2, op0=ALU.mult, op1=ALU.add)
                sdev = pt([128, 1], F32, tag="sdev", bufs=2)
                nc.scalar.activation(sdev, veps, AF.Sqrt)
                r_t = pt([128, 1], F32, tag="r_t", bufs=2)
                nc.vector.reciprocal(r_t, sdev)
                mr_t = pt([128, 1], F32, tag="mr_t", bufs=2)
                nc.vector.tensor_mul(mr_t, mu, r_t)

                # z = r*Z - (mu*r)*c + bias   on [128,48]
                z = pt([128, 48], F32, tag="z", bufs=2)
                nc.vector.tensor_scalar(out=z, in0=pz[:, 0:48], scalar1=r_t,
                                        scalar2=None, op0=ALU.mult)
                nc.vector.scalar_tensor_tensor(out=z, in0=cneg_bc, scalar=mr_t,
                                               in1=z, op0=ALU.mult, op1=ALU.add)
                nc.vector.tensor_add(z, z, bz_bc)

                # per part: top-2 softmax gates, then sum_e w_e * S_e
                ctbs = []
                for j in range(3):
                    lg = z[:, 16 * j:16 * j + 8]
                    Sv = z[:, 16 * j + 8:16 * j + 16]
                    max1 = pt([128, 1], F32, tag="max1", bufs=2)
                    nc.vector.reduce_max(max1, lg, axis=AX.X)
                    is1 = pt([128, 8], F32, tag="is1", bufs=2)
                    nc.vector.tensor_scalar(out=is1, in0=lg, scalar1=max1,
                                            scalar2=None, op0=ALU.is_equal)
                    l2 = pt([128, 8], F32, tag="l2", bufs=2)
                    nc.vector.scalar_tensor_tensor(out=l2, in0=is1, scalar=NEG_BIG,
                                                   in1=lg, op0=ALU.mult, op1=ALU.add)
                    max2 = pt([128, 1], F32, tag="max2", bufs=2)
                    nc.vector.reduce_max(max2, l2, axis=AX.X)
                    dlt = pt([128, 1], F32, tag="dlt", bufs=2)
                    nc.vector.tensor_sub(dlt, max1, max2)
                    s1 = pt([128, 1], F32, tag="s1", bufs=2)
                    nc.scalar.activation(s1, dlt, AF.Sigmoid)
                    s2 = pt([128, 1], F32, tag="s2", bufs=2)
                    nc.scalar.activation(s2, dlt, AF.Sigmoid, scale=-1.0)
                    is2 = pt([128, 8], F32, tag="is2", bufs=2)
                    nc.vector.tensor_scalar(out=is2, in0=l2, scalar1=max2,
                                            scalar2=None, op0=ALU.is_equal)
                    w_sb = pt([128, 8], F32, tag="w_sb", bufs=2)
                    nc.vector.tensor_scalar(out=w_sb, in0=is1, scalar1=s1,
                                            scalar2=None, op0=ALU.mult)
                    nc.vector.scalar_tensor_tensor(out=w_sb, in0=is2, scalar=s2,
                                                   in1=w_sb, op0=ALU.mult,
                                                   op1=ALU.add)
                    wS = pt([128, 8], F32, tag="wS", bufs=2)
                    ctb = pt([128, 1], F32, tag="ctb", bufs=3)
                    nc.vector.scalar_tensor_tensor(out=wS, in0=Sv, scalar=1.0,
                                                   in1=w_sb, op0=ALU.bypass,
                                                   op1=ALU.mult, accum_out=ctb)
                    ctbs.append(ctb)

                col = NCH * t + c
                c01 = pt([128, 1], F32, tag="c01", bufs=2)
                nc.vector.tensor_add(c01, ctbs[0], ctbs[1])
                nc.vector.tensor_add(out128[:, col:col + 1], c01, ctbs[2])

        nc.sync.dma_start(out=out_d[:, :], in_=out128)

    nc.compile()
    return nc


def _pack_vec(v, nch):
    return np.ascontiguousarray(np.asarray(v, np.float32).reshape(nch, 128).T)


def _tf32_split(w):
    """Split fp32 matrix into tf32-representable hi + lo (RNE at 11
    mantissa bits, matching the PE's fp32r rounding)."""
    w = np.ascontiguousarray(w, np.float32)

    def rnd(x):
        u = x.view(np.uint32)
        keep = ((u + 0x800 + ((u >> 12) & 1)) & 0xFFFFF000).astype(np.uint32)
        return keep.view(np.float32)

    hi = rnd(w)
    lo = rnd((w.astype(np.float64) - hi.astype(np.float64)).astype(np.float32))
    return hi, lo


def prepare_maps(inputs):
    """Host-side sharding + weight folding. Returns per-core input maps
    plus the global output constant c0."""
    f32, f64 = np.float32, np.float64
    k64 = 1.0 / np.sqrt(f64(1.0) + f64(EPS))
    g1 = np.asarray(inputs["g1"], f64)
    g2 = np.asarray(inputs["g2"], f64)

    # ---- fold proj into W1: WF_i = Wp_i @ W1_i ; b1' = sum_i bp_i@W1_i + b1
    W1 = np.asarray(inputs["W1"], f64)
    WF = np.concatenate(
        [np.asarray(inputs[f"Wp{i+1}"], f64) @ W1[D * i:D * (i + 1), :]
         for i in range(3)], axis=0)                        # [3*DIN, H]
    b1p = (np.concatenate([np.asarray(inputs[f"bp{i+1}"], f64)
                           for i in range(3)]) @ W1
           + np.asarray(inputs["b1"], f64))

    # ---- output-tail fold: out = concat(o) @ wfr + c0
    scf = np.asarray(inputs["bng"], f64) * k64
    wfr = np.asarray(inputs["Wf"], f64) @ (scf * np.asarray(inputs["Wr"], f64)[:, 0])
    c0 = ((np.asarray(inputs["bf"], f64) * scf + np.asarray(inputs["bnb"], f64))
          @ np.asarray(inputs["Wr"], f64)[:, 0] + f64(inputs["br"][0]))

    # ---- LN fold into router / expert-scalar weights
    lng = np.asarray(inputs["lng"], f64)
    lnb = np.asarray(inputs["lnb"], f64)
    Wg = np.asarray(inputs["Wg"], f64)
    bg = np.asarray(inputs["bg"], f64)
    We = np.asarray(inputs["We"], f64)
    bexp = np.asarray(inputs["bexp"], f64)
    wzfull = np.zeros((D3, 49), f64)
    cneg = np.zeros(48, f64)
    bz = np.zeros(48, f64)
    for j in range(3):
        sl = slice(D * j, D * (j + 1))
        lngj, lnbj, wfrj = lng[sl], lnb[sl], wfr[sl]
        Vj = (We @ wfrj).T                                  # [D, E]
        wzfull[sl, 16 * j:16 * j + 8] = lngj[:, None] * Wg
        wzfull[sl, 16 * j + 8:16 * j + 16] = lngj[:, None] * Vj
        cneg[16 * j:16 * j + 8] = -(lngj @ Wg)
        cneg[16 * j + 8:16 * j + 16] = -(lngj @ Vj)
        bz[16 * j:16 * j + 8] = bg + lnbj @ Wg
        bz[16 * j + 8:16 * j + 16] = bexp @ wfrj + lnbj @ Vj
    wzfull[:, 48] = 1.0

    WFhi, _ = _tf32_split(WF.astype(f32))
    W2hi, _ = _tf32_split(inputs["W2"])
    W3hi, _ = _tf32_split(inputs["W3"])
    WFlo64 = WF - WFhi.astype(f64)                      # exact lo residual
    E5 = ml_dtypes.float8_e5m2

    def _pack_pairs(arr):                               # [K,N] -> [128,K/256,2,N]
        K, N = arr.shape
        return np.ascontiguousarray(
            arr.reshape(K // 256, 2, 128, N).transpose(2, 0, 1, 3))
    # z comes entirely from h2:  Z = h2 @ (W3 @ wz) + b3 @ wz  (exact W3),
    # col 48 is the feature-sum for mu:  sum(t3) = h2 @ W3.sum(1) + sum(b3)
    W3f = np.asarray(inputs["W3"], f64)
    b3f = np.asarray(inputs["b3"], f64)
    wzh = np.zeros((H, 49), f64)
    wzh[:, 0:48] = W3f @ wzfull[:, 0:48]
    wzh[:, 48] = W3f.sum(1)
    zb = np.zeros(49, f64)
    zb[0:48] = b3f @ wzfull[:, 0:48]
    zb[48] = b3f.sum()
    consts = {
        "WFhi": WFhi,
        "W2hir": np.ascontiguousarray(
            W2hi.reshape(_chunks(H), 128, H).transpose(1, 0, 2)),
        "W3hi": W3hi,
        "WFh8": _pack_pairs((WFhi.astype(f64) / 16.0).astype(f32)).astype(E5),
        "WFl8": _pack_pairs((WFlo64 * 1024.0).astype(f32)).astype(E5),
        "W2h8": _pack_pairs((W2hi.astype(f64) / 16.0).astype(f32)).astype(E5),
        "wzh": np.ascontiguousarray(
            wzh.astype(f32).reshape(_chunks(H), 128, 49).transpose(1, 0, 2)),
        "zb": zb.astype(f32).reshape(1, 49),
        "ones_row": np.ones((1, 128), f32),
        "s1v": _pack_vec((g1 * k64).astype(f32), _chunks(H)),
        "b1v": _pack_vec((b1p * g1 * k64
                          + np.asarray(inputs["be1"], f64)).astype(f32), _chunks(H)),
        "s2v": _pack_vec((g2 * k64).astype(f32), _chunks(H)),
        "b2v": _pack_vec((np.asarray(inputs["b2"], f64) * g2 * k64
                          + np.asarray(inputs["be2"], f64)).astype(f32), _chunks(H)),
        "b3v": _pack_vec(inputs["b3"], _chunks(D3)),
        "cneg": cneg.astype(f32).reshape(1, 48),
        "bz": bz.astype(f32).reshape(1, 48),
        "ones_col": np.ones((128, 1), f32),
    }
    xts = [np.ascontiguousarray(np.asarray(inputs[f"x{i+1}"], f32).T)
           for i in range(3)]
    in_maps = []
    for c in range(N_CORES):
        m = dict(consts)
        sl = slice(c * TOK_CORE, (c + 1) * TOK_CORE)
        for i in range(3):
            m[f"x{i+1}t"] = np.ascontiguousarray(xts[i][:, sl])
        in_maps.append(m)
    return in_maps, c0


def run(inputs, trace=False, n_tok=TOK_CORE):
    key = n_tok
    if key not in _PROGRAM_CACHE:
        _PROGRAM_CACHE[key] = build_program(n_tok=n_tok)
    nc = _PROGRAM_CACHE[key]
    in_maps, c0 = prepare_maps(inputs)
    res = run_bass_kernel_spmd(nc, in_maps, list(range(N_CORES)), trace=trace)
    rows = []
    for c in range(N_CORES):
        arr = res.results[c]["out"]            # [128, NCOL]; token = col*128 + row
        rows.append(np.ascontiguousarray(arr.T).reshape(-1))
    out = (np.concatenate(rows).astype(np.float64) + c0).astype(np.float32)
    return out.reshape(B, 1), res


def kernel(**inputs):
    out, _ = run(inputs, trace=False)
    return out
